# revision 1
# baseline (speedup 1.0000x reference)
"""DiT-X MoE block (top-2 of 4 experts + shared FFN) on 8 trn2 NeuronCores.

Strategy (data-parallel over batch, per the sharding hint):
  * B=8 samples -> one sample per NeuronCore. Routing is per-sample, so the
    tiny gate network (a few 1e5-FLOP matmuls on (B, 3D) aggregates) is
    evaluated on the host, which then ships to each core ONLY the weights of
    its two active experts plus the shared FFN. The device runs a dense,
    static 3-FFN pipeline per sample -- the top-2 sparsity is realized at
    shard time, no data-dependent control flow on device.
  * All matmuls run in bf16 (weights+activations cast on host / on chip) with
    fp32 PSUM accumulation; per-token combine weights (gate weight x modality
    mask) are applied to the hidden activations in fp32 before the second
    matmul, so the three expert contributions accumulate in a single fp32
    PSUM chain per output tile.
  * Activation layout is transposed (channels on partitions, tokens on the
    free dim) so both matmuls consume natural-layout weight tiles as the
    stationary operand and no on-device transpose is needed anywhere.

Shapes (fixed): B=8, L=768, D=1024, H=4096, E=4, K=2.
"""

import numpy as np
import ml_dtypes

B, L, D, H = 8, 768, 1024, 4096
NUM_EXPERTS, TOP_K = 4, 2
L3 = L // 3  # head / wrist / proprio segment length
KD = D // 128  # 8   k-tiles over D
KH = H // 128  # 32  k-tiles over H
# token-dim matmul chunks (PSUM bank limit: 512 fp32 per matmul)
CHUNKS = ((0, 512), (512, 256))

BF16 = ml_dtypes.bfloat16

_NC_CACHE = {}


def _gate_host(context_c, time_cond, gate_w, gate_b, time_w, time_b):
    """Replicates the reference gating math in fp32 numpy.

    Returns (topk_idx (B,2) int, topk_w (B,2) f32)."""
    full_agg = context_c.mean(axis=1)
    hp_agg = np.concatenate(
        [context_c[:, :L3], context_c[:, 2 * L3 :]], axis=1
    ).mean(axis=1)
    wp_agg = context_c[:, L3:].mean(axis=1)
    gate_in = np.concatenate([full_agg, hp_agg, wp_agg], axis=-1)

    logits = gate_in @ gate_w + gate_b
    silu = time_cond / (1.0 + np.exp(-time_cond))
    mod = silu @ time_w + time_b
    scale, shift = mod[:, :NUM_EXPERTS], mod[:, NUM_EXPERTS:]
    logits = logits * (1.0 + scale) + shift

    z = np.exp(logits - logits.max(axis=-1, keepdims=True))
    scores = z / z.sum(axis=-1, keepdims=True)

    # top-2, ties resolved to the lower index (jax.lax.top_k semantics)
    idx = np.argsort(-scores, axis=-1, kind="stable")[:, :TOP_K]
    w = np.take_along_axis(scores, idx, axis=-1)
    w = w / (w.sum(axis=-1, keepdims=True) + 1e-8)
    return idx, w.astype(np.float32)


def _modality_mask():
    mask = np.ones((NUM_EXPERTS, L), dtype=np.float32)
    mask[1, L3 : 2 * L3] = 0.0  # expert 1 skips wrist
    mask[2, :L3] = 0.0          # expert 2 skips head
    return mask


def _build_nc(act="Gelu_apprx_tanh", with_b1=False, repeat=1, stream_weights=True, do_post=True, dedupe=True, chunks=None):
    import concourse.mybir as mybir
    import concourse.tile as tile
    from concourse import bacc
    from contextlib import ExitStack

    f32 = mybir.dt.float32
    bf16 = mybir.dt.bfloat16
    GELU = getattr(mybir.ActivationFunctionType, act)

    nc = bacc.Bacc(None, target_bir_lowering=False)
    # Per-core inputs, pre-permuted on host so every DMA is per-partition
    # contiguous:
    #   xt:  [p, ko, t]        = x.T tiles       (ko over D)
    #   w1:  [j, m, p, ko, f]  = W1[j][ko*128+p, m*128+f]   (K=D stationary)
    #   w2:  [j, d, p, ko, f]  = W2[j][ko*128+p, d*128+f]   (K=H stationary)
    #   wrep:[p, j, t]         combine weight per token, replicated across p
    #   b1:  [p, j, m]         first-layer bias per H channel
    xt_d = nc.declare_dram_parameter("xt", [128, KD, L], bf16, isOutput=False)
    w1_d = nc.declare_dram_parameter("w1", [3, KH, 128, KD, 128], bf16, isOutput=False)
    w2_d = nc.declare_dram_parameter("w2", [3, KD, 128, KH, 128], bf16, isOutput=False)
    wrep_d = nc.declare_dram_parameter("wrep", [128, 2, L], f32, isOutput=False)
    b1_d = None
    if with_b1:
        b1_d = nc.declare_dram_parameter("b1", [128, 3, KH], f32, isOutput=False)
    y_d = nc.declare_dram_parameter("y", [128, KD, L], f32, isOutput=True)

    with tile.TileContext(nc) as tc, ExitStack() as ctx:
        const = ctx.enter_context(tc.tile_pool(name="const", bufs=1))
        w1p = ctx.enter_context(tc.tile_pool(name="w1p", bufs=3))
        w2p = ctx.enter_context(tc.tile_pool(name="w2p", bufs=2))
        hp = ctx.enter_context(tc.tile_pool(name="hp", bufs=2))
        gp = ctx.enter_context(tc.tile_pool(name="gp", bufs=3))
        op = ctx.enter_context(tc.tile_pool(name="op", bufs=3))
        psA = ctx.enter_context(tc.tile_pool(name="psA", bufs=2, space="PSUM"))
        psB = ctx.enter_context(tc.tile_pool(name="psB", bufs=2, space="PSUM"))

        xt = const.tile([128, KD, L], bf16)
        nc.sync.dma_start(xt, xt_d[:])
        wrep = const.tile([128, 2, L], f32)
        nc.sync.dma_start(wrep, wrep_d[:])
        b1 = None
        if with_b1:
            b1 = const.tile([128, 3, KH], f32)
            nc.sync.dma_start(b1, b1_d[:])
        res_w1 = res_w2 = None
        if not stream_weights:
            # microbench mode: one resident weight tile reused for all matmuls
            res_w1 = const.tile([128, KD, 128], bf16, tag="res_w1")
            nc.sync.dma_start(res_w1, w1_d[0, 0])
            res_w2 = const.tile([128, KH, 128], bf16, tag="res_w2")
            nc.sync.dma_start(res_w2, w2_d[0, 0])
        ch = CHUNKS if chunks is None else chunks
        for _rep in range(repeat):
            _emit_body(nc, tc, mybir, GELU, ctx, const, w1p, w2p, hp, gp, op, psA, psB,
                       xt, wrep, b1, w1_d, w2_d, y_d, with_b1, res_w1, res_w2, do_post, ch)

    nc.compile()
    if dedupe:
        _dedupe_ldweights(nc, mybir)
    return nc


def _dedupe_ldweights(nc, mybir):
    """Drop an InstLdweights whose weights AP equals the immediately
    preceding PE weight load -- the stationary operand is still resident in
    the array, so the reload is pure overhead (~50ns each, ~1500 per pass).
    Only sync-free duplicates are dropped; anything carrying waits/updates,
    or following a non-LDW/MM PE instruction, is kept."""
    PE = mybir.EngineType.PE
    dropped = 0
    for fn in nc.m.functions:
        for bb in fn.blocks:
            insts = bb.instructions
            keep = []
            prev_key = None
            for ins in insts:
                if ins.engine != PE:
                    keep.append(ins)
                    continue
                t = type(ins).__name__
                if t == "InstLdweights":
                    key = repr(ins.ins[0])
                    si = ins.sync_info
                    clean = not si or (not si.on_wait and not si.on_update)
                    if key == prev_key and clean:
                        dropped += 1
                        continue
                    prev_key = key
                    keep.append(ins)
                elif t == "InstMatmult":
                    keep.append(ins)
                else:
                    prev_key = None  # barrier/drain/branch: be conservative
                    keep.append(ins)
            if dropped and len(keep) != len(insts):
                bb.instructions = keep
    nc._dedupe_ldw_dropped = dropped
    return dropped


def _emit_body(nc, tc, mybir, GELU, ctx, const, w1p, w2p, hp, gp, op, psA, psB,
               xt, wrep, b1, w1_d, w2_d, y_d, with_b1, res_w1=None, res_w2=None, do_post=True,
               ch=CHUNKS):
    import concourse.tile as tile  # noqa
    f32 = mybir.dt.float32
    bf16 = mybir.dt.bfloat16
    if True:
        acc = const.tile([128, KD, L], f32, tag="acc")

        for j in range(3):  # expert slot 0, expert slot 1, shared
            # ---- first layer: hj[p_H, m, t] = gelu(x @ W1j + b1j) [* wvec_j]
            hj = hp.tile([128, KH, L], bf16, tag="hj", name="hj") if do_post else None
            for m in range(KH):
                if res_w1 is not None:
                    w1t = res_w1
                else:
                    w1t = w1p.tile([128, KD, 128], bf16, tag="w1t")
                    nc.sync.dma_start(w1t, w1_d[j, m])
                hps = psA.tile([128, ch[0][1]], f32, tag="hps")
                hps2 = psA.tile([128, ch[1][1]], f32, tag="hps2")
                for ci, ((off, n), ps) in enumerate(zip(ch, (hps, hps2))):
                    # snake the k order so the chunk boundary reuses the
                    # resident weights (the duplicate LDW is deduped below)
                    ks = list(range(KD)) if ci == 0 else list(range(KD - 1, -1, -1))
                    for ki, k in enumerate(ks):
                        nc.tensor.matmul(
                            ps[:, :n],
                            w1t[:, k, :],
                            xt[:, k, off : off + n],
                            start=(ki == 0),
                            stop=(ki == KD - 1),
                        )
                if not do_post:
                    continue
                if with_b1:
                    # generic path: add the (rarely nonzero) first-layer bias
                    # on DVE before the activation; the HW ACT instruction has
                    # too few sync-wait slots to take the bias AP directly.
                    for (off, n), ps in zip(ch, (hps, hps2)):
                        nc.vector.tensor_scalar_add(ps[:, :n], ps[:, :n], b1[:, j, m : m + 1])
                if j < 2:
                    g = gp.tile([128, L], f32, tag="g")
                    for (off, n), ps in zip(ch, (hps, hps2)):
                        nc.scalar.activation(g[:, off : off + n], ps[:, :n], GELU)
                    nc.vector.tensor_mul(hj[:, m, :], g, wrep[:, j, :])
                else:
                    for (off, n), ps in zip(ch, (hps, hps2)):
                        nc.scalar.activation(hj[:, m, off : off + n], ps[:, :n], GELU)

            # ---- second layer: y[p_D, d, t] (+)= hj @ W2j
            for d in range(KD):
                if res_w2 is not None:
                    w2t = res_w2
                else:
                    w2t = w2p.tile([128, KH, 128], bf16, tag="w2t")
                    nc.sync.dma_start(w2t, w2_d[j, d])
                yps = psB.tile([128, ch[0][1]], f32, tag="yps")
                yps2 = psB.tile([128, ch[1][1]], f32, tag="yps2")
                for ci, ((off, n), ps) in enumerate(zip(ch, (yps, yps2))):
                    ks = list(range(KH)) if ci == 0 else list(range(KH - 1, -1, -1))
                    for ki, k in enumerate(ks):
                        rhs2 = hj[:, k, off : off + n] if do_post else xt[:, k % KD, off : off + n]
                        nc.tensor.matmul(
                            ps[:, :n],
                            w2t[:, k, :],
                            rhs2,
                            start=(ki == 0),
                            stop=(ki == KH - 1),
                        )
                if not do_post:
                    continue
                if j == 0:
                    for (off, n), ps in zip(ch, (yps, yps2)):
                        nc.vector.tensor_copy(acc[:, d, off : off + n], ps[:, :n])
                elif j == 1:
                    for (off, n), ps in zip(ch, (yps, yps2)):
                        nc.vector.tensor_add(
                            acc[:, d, off : off + n], acc[:, d, off : off + n], ps[:, :n]
                        )
                else:
                    ot = op.tile([128, L], f32, tag="ot")
                    for (off, n), ps in zip(ch, (yps, yps2)):
                        nc.vector.tensor_add(
                            ot[:, off : off + n], acc[:, d, off : off + n], ps[:, :n]
                        )
                    nc.sync.dma_start(y_d[:, d, :], ot)


def _get_nc(with_b1=False):
    key = ("nc", with_b1)
    if key not in _NC_CACHE:
        _NC_CACHE[key] = _build_nc(with_b1=with_b1)
    return _NC_CACHE[key]


def _pack_core_inputs(x, e0, e1, w0, w1, ew1, ew2, sw1, sw2, eb1, sb1, mask, with_b1=False):
    """Build the per-core input dict (all layouts per-partition contiguous)."""
    # x: (L, D) fp32 -> xt [p, ko, t] bf16
    xT = np.ascontiguousarray(x.T)  # (D, L)
    xt = np.ascontiguousarray(
        xT.reshape(KD, 128, L).transpose(1, 0, 2)
    ).astype(BF16)

    # W1 stack (3, D, H) -> [j, m, p, ko, f]
    w1s = np.stack([ew1[e0], ew1[e1], sw1])
    w1t = np.ascontiguousarray(
        w1s.reshape(3, KD, 128, KH, 128).transpose(0, 3, 2, 1, 4)
    ).astype(BF16)

    # W2 stack (3, H, D) -> [j, d, p, ko, f]
    w2s = np.stack([ew2[e0], ew2[e1], sw2])
    w2t = np.ascontiguousarray(
        w2s.reshape(3, KH, 128, KD, 128).transpose(0, 3, 2, 1, 4)
    ).astype(BF16)

    # combine weights (per token), replicated over partitions
    wvec = np.stack([w0 * mask[e0], w1 * mask[e1]]).astype(np.float32)  # (2, L)
    wrep = np.ascontiguousarray(
        np.broadcast_to(wvec[None], (128, 2, L))
    ).astype(np.float32)

    out = {"xt": xt, "w1": w1t, "w2": w2t, "wrep": wrep}
    if with_b1:
        # first-layer biases [p, j, m]
        b1s = np.stack([eb1[e0], eb1[e1], sb1]).astype(np.float32)  # (3, H)
        out["b1"] = np.ascontiguousarray(
            b1s.reshape(3, KH, 128).transpose(2, 0, 1)
        ).astype(np.float32)
    return out


def kernel(
    context_c,
    time_cond,
    gate_w,
    gate_b,
    time_w,
    time_b,
    ew1,
    eb1,
    ew2,
    eb2,
    sw1,
    sb1,
    sw2,
    sb2,
):
    from concourse.bass_utils import run_bass_kernel_spmd

    context_c = np.asarray(context_c, dtype=np.float32)
    time_cond = np.asarray(time_cond, dtype=np.float32)

    topk_idx, topk_w = _gate_host(
        context_c, time_cond,
        np.asarray(gate_w, np.float32), np.asarray(gate_b, np.float32),
        np.asarray(time_w, np.float32), np.asarray(time_b, np.float32),
    )
    mask = _modality_mask()
    eb1 = np.asarray(eb1, np.float32)
    sb1 = np.asarray(sb1, np.float32)
    with_b1 = bool(np.any(eb1) or np.any(sb1))

    ew1 = np.asarray(ew1, np.float32)
    ew2 = np.asarray(ew2, np.float32)
    sw1 = np.asarray(sw1, np.float32)
    sw2 = np.asarray(sw2, np.float32)

    in_maps = []
    for b in range(B):
        e0, e1 = int(topk_idx[b, 0]), int(topk_idx[b, 1])
        in_maps.append(
            _pack_core_inputs(
                context_c[b], e0, e1, topk_w[b, 0], topk_w[b, 1],
                ew1, ew2, sw1, sw2,
                eb1, sb1, mask, with_b1=with_b1,
            )
        )

    nc = _get_nc(with_b1=with_b1)
    _NC_CACHE["last_in_maps"] = in_maps
    res = run_bass_kernel_spmd(nc, in_maps, core_ids=list(range(B)))

    eb2 = np.asarray(eb2, np.float32)
    sb2 = np.asarray(sb2, np.float32)
    out = np.empty((B, L, D), np.float32)
    for b in range(B):
        y = res.results[b]["y"]  # [p, d, t]
        out[b] = y.transpose(2, 1, 0).reshape(L, D)
        # second-layer biases are additive at the output; fold on host
        e0, e1 = int(topk_idx[b, 0]), int(topk_idx[b, 1])
        wv0 = topk_w[b, 0] * mask[e0]
        wv1 = topk_w[b, 1] * mask[e1]
        out[b] += (
            wv0[:, None] * eb2[e0][None, :]
            + wv1[:, None] * eb2[e1][None, :]
            + sb2[None, :]
        )
    return out

